# revision 1
# baseline (speedup 1.0000x reference)
"""Trainium2 Bass kernel for fused single-head attention.

Problem: x:(4,4096,256) f32; q/k/v = x@W*.T + b*; out = softmax(q@k.T/16) @ v.

Sharding: 8 cores = 4 batches x 2 query-halves. Each core holds full K/V for
its batch (recomputed per core-pair; proj is cheap) and half the queries.

Per-core algorithm (all matmuls in float32r: full PE rate at N>=256,
~12-bit mantissa):
  xT[d,s]   : x[b].T with this core's query half rotated to columns 0:2048
              (a key-permutation is softmax-invariant, so one SPMD program
              serves both halves).
  QT[e,q]   = wqT.T @ xT[:, :2048] + bq   (wqT, bq pre-scaled by 1/sqrt(D))
  KT[e,k]   = wkT.T @ xT + bk
  V[k,e]    = xT.T @ wvT                  (bv folded in after softmax)
  ST[k,q]   = KT.T @ QT                   (scores, transposed: k on partitions)
  ET[k,q]   = exp(ST)                     (no max-subtraction: |scores| < ~10)
  acc[q,e+1]= ET.T @ [V | 1]              (ones column -> softmax denominators)
  out[q,e]  = acc[:, :256] / acc[:, 256] + bv
"""

import numpy as np

import concourse.bass as bass
import concourse.mybir as mybir
import concourse.tile as tile
from concourse import bacc
from concourse.bass_utils import run_bass_kernel_spmd

B, S, D = 4, 4096, 256
N_CORES = 8
SQ = S // 2          # queries per core
F32 = mybir.dt.float32
F32R = mybir.dt.float32r
EXP = mybir.ActivationFunctionType.Exp
# dtype for the exp'd scores (lhsT of the output matmul) and V. HW-calibrated
# at the B-phase shape (N=258, weight swap every MM): bf16 = ~68 ns/MM vs
# f32r = ~138 ns/MM -- bf16's FWL weight load hides where f32r's self-load
# does not. Costs ~1.7e-3 of scale-relative error (still >>10x under gate).
BF16 = mybir.dt.bfloat16
ET_DT = BF16

N_QB = SQ // 512     # 4 query blocks of 512
N_KT = S // 128      # 32 key tiles of 128


def _build(repeats=1):
    nc = bacc.Bacc("TRN2", target_bir_lowering=False, debug=False,
                   enable_asserts=False, num_devices=N_CORES)

    xT = nc.dram_tensor("xT", [D, S], F32R, kind="ExternalInput").ap()
    # packed [WvT | WqT*scale | WkT] along columns: [D, 3*D]
    wall = nc.dram_tensor("wall", [D, 3 * D], F32R, kind="ExternalInput").ap()
    bq2 = nc.dram_tensor("bq2", [128, 2], F32, kind="ExternalInput").ap()
    bk2 = nc.dram_tensor("bk2", [128, 2], F32, kind="ExternalInput").ap()
    # [bv(256) | vpad-row(128: 32 x {1,0,0,0})]
    smalls = nc.dram_tensor("smalls", [1, D + 128], F32, kind="ExternalInput").ap()
    out = nc.dram_tensor("out", [SQ, D], F32, kind="ExternalOutput").ap()

    with tile.TileContext(nc) as tc:
        for _rep in range(repeats):
            _emit_body(nc, tc, _rep, xT, wall, bq2, bk2, smalls, out)

    nc.compile()
    return nc


def _emit_body(nc, tc, rep, xT, wall, bq2, bk2, smalls, out):
        with (
            tc.tile_pool(name=f"persist{rep}", bufs=1) as persist,
            tc.tile_pool(name=f"ets{rep}", bufs=36) as ets,
            tc.tile_pool(name=f"outs{rep}", bufs=3) as outs,
            tc.tile_pool(name=f"small{rep}", bufs=4) as small,
            tc.tile_pool(name=f"pp{rep}", bufs=3, space="PSUM") as pp,
            tc.tile_pool(name=f"pss{rep}", bufs=3, space="PSUM") as pss,
            tc.tile_pool(name=f"pso{rep}", bufs=2, space="PSUM") as pso,
        ):
            # ---- persistent SBUF ----
            xt_sb = persist.tile([128, 2, S], F32R)       # x[b].T, d-chunked
            w_sb = persist.tile([128, 2, 3 * D], F32R)    # [wv|wq|wk], d-chunked
            qt_sb = persist.tile([128, 2, SQ], F32R)      # QT, e-chunked
            kt_sb = persist.tile([128, 2, S], F32R)       # KT, e-chunked
            # [V | 1 | 0 0 0] k-chunked (f32r matmul dst needs even width)
            vaug_sb = persist.tile([128, N_KT, D + 2], ET_DT)
            bq_sb = persist.tile([128, 2], F32)
            bk_sb = persist.tile([128, 2], F32)
            bv_sb = persist.tile([128, D], F32)

            # d-chunk 0 streams on the sync HWDGE ring, d-chunk 1 on the
            # scalar ring: both rings drain concurrently, so projections can
            # start ~3us in instead of waiting for the full 4.75MB load.
            ring = [nc.sync, nc.scalar]
            sm_sb = persist.tile([1, D + 128], F32)
            nc.sync.dma_start(bq_sb[:], bq2[:])
            nc.scalar.dma_start(bk_sb[:], bk2[:])
            for c in range(2):
                ring[c].dma_start(w_sb[:, c, 0:D],
                                  wall[128 * c:128 * (c + 1), 0:D])
            for blk in range(8):
                for c in range(2):
                    ring[c].dma_start(
                        xt_sb[:, c, 512 * blk:512 * (blk + 1)],
                        xT[128 * c:128 * (c + 1), 512 * blk:512 * (blk + 1)])
                if blk == 0:
                    for c in range(2):
                        ring[c].dma_start(w_sb[:, c, D:3 * D],
                                          wall[128 * c:128 * (c + 1), D:3 * D])
                elif blk == 3:
                    nc.scalar.dma_start(sm_sb[:], smalls[:])

            # PE warm-up: dummy matmuls on a memset tile (no DMA dep) while
            # the input stream lands -- flips the HAM clock-gate to 2.4 GHz
            # and keeps the PE busy through the initial load.
            ones_sb = persist.tile([1, 128], F32)
            nc.vector.memset(ones_sb[:], 1.0)
            warm = pp.tile([128, 512], F32, tag="pp")
            for _w in range(6):
                nc.tensor.matmul(warm[:, 0:128], ones_sb[:],
                                 ones_sb[:], start=True, stop=True)

            # ---- projections (column-block outer: start on first xT blk) ----
            for blk in range(8):
                # V[k,e] = sum_d xT[d,k] * wvT[d,e]  (no bias dep: first)
                for kc in range(4 * blk, 4 * blk + 4):
                    pv = pp.tile([128, 512], F32, tag="pp")
                    for dc in range(2):
                        nc.tensor.matmul(
                            pv[:, 0:D],
                            xt_sb[:, dc, 128 * kc:128 * (kc + 1)],
                            w_sb[:, dc, 0:D],
                            start=(dc == 0), stop=(dc == 1))
                    nc.vector.tensor_copy(vaug_sb[:, kc, 0:D], pv[:, 0:D])
                # QT[e,q]: queries are xT columns 0:SQ (blocks 0..3)
                if blk < 4:
                    for ec in range(2):
                        pq = pp.tile([128, 512], F32, tag="pp")
                        for dc in range(2):
                            nc.tensor.matmul(
                                pq[:],
                                w_sb[:, dc, D + 128 * ec:D + 128 * (ec + 1)],
                                xt_sb[:, dc, 512 * blk:512 * (blk + 1)],
                                start=(dc == 0), stop=(dc == 1))
                        nc.vector.tensor_scalar_add(
                            qt_sb[:, ec, 512 * blk:512 * (blk + 1)], pq[:],
                            bq_sb[:, ec:ec + 1])
                # KT[e,k] = sum_d wkT[d,e] * xT[d,k]  (+bk)
                for ec in range(2):
                    pk = pp.tile([128, 512], F32, tag="pp")
                    for dc in range(2):
                        nc.tensor.matmul(
                            pk[:],
                            w_sb[:, dc, 2 * D + 128 * ec:2 * D + 128 * (ec + 1)],
                            xt_sb[:, dc, 512 * blk:512 * (blk + 1)],
                            start=(dc == 0), stop=(dc == 1))
                    nc.vector.tensor_scalar_add(
                        kt_sb[:, ec, 512 * blk:512 * (blk + 1)], pk[:],
                        bk_sb[:, ec:ec + 1])

            # broadcast bv / vaug pad columns across partitions via K=1
            # matmuls against a ones row (emitted after the projections so
            # they don't head-block the PE FIFO waiting for the smalls DMA)
            pbv = pso.tile([128, D + 2], F32, tag="po")
            nc.tensor.matmul(pbv[:, 0:D], ones_sb[:], sm_sb[:, 0:D],
                             start=True, stop=True)
            nc.vector.tensor_copy(bv_sb[:], pbv[:, 0:D])
            ppad = pso.tile([128, D + 2], F32, tag="po")
            nc.tensor.matmul(ppad[:, 0:128], ones_sb[:], sm_sb[:, D:D + 128],
                             start=True, stop=True)
            nc.vector.tensor_copy(vaug_sb[:, :, D:D + 2], ppad[:, 0:64])

            # ---- attention ----
            for qb in range(N_QB):
                et_tiles = []
                for kt in range(N_KT):
                    ps = pss.tile([128, 512], F32, tag="pss")
                    for ec in range(2):
                        nc.tensor.matmul(
                            ps[:],
                            kt_sb[:, ec, 128 * kt:128 * (kt + 1)],
                            qt_sb[:, ec, 512 * qb:512 * (qb + 1)],
                            start=(ec == 0), stop=(ec == 1))
                    et = ets.tile([128, 512], ET_DT, tag="et",
                                  name=f"et_{qb}_{kt}")
                    nc.scalar.activation(et[:], ps[:], EXP)
                    et_tiles.append(et)
                for qs in range(4):
                    po = pso.tile([128, D + 2], F32, tag="po")
                    for kc in range(N_KT):
                        nc.tensor.matmul(
                            po[:],
                            et_tiles[kc][:, 128 * qs:128 * (qs + 1)],
                            vaug_sb[:, kc, :],
                            start=(kc == 0), stop=(kc == N_KT - 1))
                    rec = small.tile([128, 1], F32, tag="rec")
                    nc.vector.reciprocal(rec[:], po[:, D:D + 1])
                    ot = outs.tile([128, D], F32, tag="ot")
                    nc.vector.scalar_tensor_tensor(
                        ot[:], po[:, 0:D], rec[:], bv_sb[:],
                        mybir.AluOpType.mult, mybir.AluOpType.add)
                    q0 = (qb * 4 + qs) * 128
                    nc.sync.dma_start(out[q0:q0 + 128, :], ot[:])


_NC = None


def _get_nc():
    global _NC
    if _NC is None:
        _NC = _build()
    return _NC


def round_f32r(x: np.ndarray) -> np.ndarray:
    """Round fp32 to fp32r (12-bit mantissa) to match on-chip rounding."""
    u = x.astype(np.float32).view(np.uint32)
    u = ((u.astype(np.uint64) + 0x400) & 0xFFFFF800).astype(np.uint32)
    return u.view(np.float32)


def make_in_maps(x, Wq, bq, Wk, bk, Wv, bv):
    x = np.asarray(x, dtype=np.float32)
    Wq = np.asarray(Wq, dtype=np.float32)
    bq = np.asarray(bq, dtype=np.float32)
    Wk = np.asarray(Wk, dtype=np.float32)
    bk = np.asarray(bk, dtype=np.float32)
    Wv = np.asarray(Wv, dtype=np.float32)
    bv = np.asarray(bv, dtype=np.float32)

    scale = 1.0 / np.sqrt(np.float32(D))
    wall = round_f32r(np.concatenate(
        [Wv.T, Wq.T * scale, Wk.T], axis=1))  # [d_in, 3*e_out]
    bq2 = np.ascontiguousarray((bq * scale).reshape(2, 128).T)
    bk2 = np.ascontiguousarray(bk.reshape(2, 128).T)
    vpad_row = np.zeros(64, dtype=np.float32)
    vpad_row[0::2] = 1.0
    smalls = np.concatenate([bv, vpad_row, np.zeros(64, np.float32)]).reshape(1, D + 128)
    smalls = np.ascontiguousarray(smalls.astype(np.float32))

    in_maps = []
    for c in range(N_CORES):
        b, h = divmod(c, 2)
        xt = x[b].T  # [D, S]
        # rotate this core's query half to columns 0:SQ (k-perm is invariant)
        xt = np.concatenate([xt[:, h * SQ:(h + 1) * SQ],
                             xt[:, (1 - h) * SQ:(2 - h) * SQ]], axis=1)
        in_maps.append({
            "xT": round_f32r(np.ascontiguousarray(xt)),
            "wall": wall,
            "bq2": bq2, "bk2": bk2, "smalls": smalls,
        })
    return in_maps


def kernel(x, Wq, bq, Wk, bk, Wv, bv):
    in_maps = make_in_maps(x, Wq, bq, Wk, bk, Wv, bv)
    nc = _get_nc()
    res = run_bass_kernel_spmd(nc, in_maps, core_ids=list(range(N_CORES)))

    full = np.empty((B, S, D), dtype=np.float32)
    for c in range(N_CORES):
        b, h = divmod(c, 2)
        full[b, h * SQ:(h + 1) * SQ, :] = res.results[c]["out"]
    return full



# revision 7
# speedup vs baseline: 1.9187x; 1.9187x over previous
"""Trainium2 Bass kernel for fused single-head attention.

Problem: x:(4,4096,256) f32; q/k/v = x@W*.T + b*; out = softmax(q@k.T/16) @ v.

Sharding: 8 cores = 4 batches x 2 query-halves. Each core holds full K/V for
its batch (recomputed per core-pair; proj is cheap) and half the queries.

Per-core algorithm (all matmuls in float32r: full PE rate at N>=256,
~12-bit mantissa):
  xT[d,s]   : x[b].T with this core's query half rotated to columns 0:2048
              (a key-permutation is softmax-invariant, so one SPMD program
              serves both halves).
  QT[e,q]   = wqT.T @ xT[:, :2048] + bq   (wqT, bq pre-scaled by 1/sqrt(D))
  KT[e,k]   = wkT.T @ xT + bk
  V[k,e]    = xT.T @ wvT                  (bv folded in after softmax)
  ST[k,q]   = KT.T @ QT                   (scores, transposed: k on partitions)
  ET[k,q]   = exp(ST)                     (no max-subtraction: |scores| < ~10)
  acc[q,e+1]= ET.T @ [V | 1]              (ones column -> softmax denominators)
  out[q,e]  = acc[:, :256] / acc[:, 256] + bv

Schedule (v2): PE is the bottleneck engine (~127us of matmul at 2.4GHz), so
the emission order keeps it dense:
  - the AV (output) matmuls of query-block qb-1 are interleaved between the
    scores matmuls of qb, so the PE never stalls waiting for the ACT engine
    to exp a scores tile (exp of a 512-col tile takes 720ns vs 427ns to
    produce it);
  - exp runs on 1024-wide PSUM tiles (2 key-tiles per instruction) to halve
    the ACT fixed overhead;
  - projection-phase elementwise work is split: V copies + Q bias on ACT,
    K bias on DVE;
  - input DMA streams on the sync + gpsimd rings (both otherwise idle).
"""

import numpy as np

import concourse.bass as bass
import concourse.mybir as mybir
import concourse.tile as tile
from concourse import bacc
from concourse.bass_utils import run_bass_kernel_spmd

B, S, D = 4, 4096, 256
N_CORES = 8
SQ = S // 2          # queries per core
F32 = mybir.dt.float32
F32R = mybir.dt.float32r
EXP = mybir.ActivationFunctionType.Exp
COPY = mybir.ActivationFunctionType.Copy
# dtype for the exp'd scores (lhsT of the output matmul) and V. bf16 matmul
# gets FWL weight load (hidden behind the moving stream) where f32r's
# self-load does not. Costs ~1.7e-3 of scale-relative error (>>10x under gate).
BF16 = mybir.dt.bfloat16
ET_DT = BF16

N_QB = SQ // 512     # 4 query blocks of 512
N_KT = S // 128      # 32 key tiles of 128


def _build(repeats=1):
    nc = bacc.Bacc("TRN2", target_bir_lowering=False, debug=False,
                   enable_asserts=False, num_devices=N_CORES)

    xT = nc.dram_tensor("xT", [D, S], F32R, kind="ExternalInput").ap()
    # packed [WvT | WqT*scale | WkT] along columns: [D, 3*D]
    wall = nc.dram_tensor("wall", [D, 3 * D], F32R, kind="ExternalInput").ap()
    bq2 = nc.dram_tensor("bq2", [128, 2], F32, kind="ExternalInput").ap()
    bk2 = nc.dram_tensor("bk2", [128, 2], F32, kind="ExternalInput").ap()
    # [bv(256) | vpad-row(128: 32 x {1,0,0,0})]
    smalls = nc.dram_tensor("smalls", [1, D + 128], F32, kind="ExternalInput").ap()
    out = nc.dram_tensor("out", [SQ, D], F32, kind="ExternalOutput").ap()

    with tile.TileContext(nc) as tc:
        for _rep in range(repeats):
            _emit_body(nc, tc, _rep, xT, wall, bq2, bk2, smalls, out)

    nc.compile()
    return nc


def _emit_body(nc, tc, rep, xT, wall, bq2, bk2, smalls, out):
        with (
            tc.tile_pool(name=f"persist{rep}", bufs=1) as persist,
            tc.tile_pool(name=f"ets{rep}", bufs=34) as ets,
            tc.tile_pool(name=f"outs{rep}", bufs=3) as outs,
            tc.tile_pool(name=f"small{rep}", bufs=4) as small,
            tc.tile_pool(name=f"pp{rep}", bufs=2, space="PSUM") as pp,
            tc.tile_pool(name=f"pss{rep}", bufs=2, space="PSUM") as pss,
            tc.tile_pool(name=f"pso{rep}", bufs=2, space="PSUM") as pso,
        ):
            # ---- persistent SBUF ----
            xt_sb = persist.tile([128, 2, S], F32R)       # x[b].T, d-chunked
            w_sb = persist.tile([128, 2, 3 * D], F32R)    # [wv|wq|wk], d-chunked
            qt_sb = persist.tile([128, 2, SQ], F32R)      # QT, e-chunked
            kt_sb = persist.tile([128, 2, S], F32R)       # KT, e-chunked
            # [V | 1 | 0 0 0] k-chunked
            vaug_sb = persist.tile([128, N_KT, D + 2], ET_DT)
            bq_sb = persist.tile([128, 2], F32)
            bk_sb = persist.tile([128, 2], F32)
            bv_sb = persist.tile([128, D], F32)
            sm_sb = persist.tile([1, D + 128], F32)

            # ---- input DMA: sync ring = wv + even x blocks; gpsimd ring =
            # small tensors, wk, wq, odd x blocks. Both engines are otherwise
            # idle, so neither ring steals time from compute engines. The
            # first x block streams in 128-col pieces so the first V matmul
            # can start ~1.8us in.
            rs, rg = nc.sync, nc.gpsimd
            rg.dma_start(bq_sb[:], bq2[:])
            rg.dma_start(bk_sb[:], bk2[:])
            rg.dma_start(sm_sb[:], smalls[:])
            for c in range(2):
                rs.dma_start(w_sb[:, c, 0:D], wall[128 * c:128 * (c + 1), 0:D])
            for c in range(2):
                rg.dma_start(w_sb[:, c, 2 * D:3 * D],
                             wall[128 * c:128 * (c + 1), 2 * D:3 * D])
            for kc in range(4):
                for c in range(2):
                    rs.dma_start(
                        xt_sb[:, c, 128 * kc:128 * (kc + 1)],
                        xT[128 * c:128 * (c + 1), 128 * kc:128 * (kc + 1)])
            for c in range(2):
                rg.dma_start(w_sb[:, c, D:2 * D],
                             wall[128 * c:128 * (c + 1), D:2 * D])
            for blk in range(2, 8, 2):
                for c in range(2):
                    rs.dma_start(
                        xt_sb[:, c, 512 * blk:512 * (blk + 1)],
                        xT[128 * c:128 * (c + 1), 512 * blk:512 * (blk + 1)])
            for blk in range(1, 8, 2):
                for c in range(2):
                    rg.dma_start(
                        xt_sb[:, c, 512 * blk:512 * (blk + 1)],
                        xT[128 * c:128 * (c + 1), 512 * blk:512 * (blk + 1)])

            # PE warm-up: dummy matmuls on a memset tile (no DMA dep) while
            # the input stream lands -- flips the HAM clock-gate to 2.4 GHz
            # and keeps the PE busy through the initial load.
            ones_sb = persist.tile([1, 128], F32)
            nc.vector.memset(ones_sb[:], 1.0)
            warm = pp.tile([128, 512], F32, tag="pp")
            for _w in range(6):
                nc.tensor.matmul(warm[:, 0:128], ones_sb[:],
                                 ones_sb[:], start=True, stop=True)

            # broadcast bv / vaug pad columns across partitions via K=1
            # matmuls against a ones row (smalls is at the head of the gpsimd
            # ring, so these run during the initial x load)
            pbv = pso.tile([128, D + 2], F32, tag="po")
            nc.tensor.matmul(pbv[:, 0:D], ones_sb[:], sm_sb[:, 0:D],
                             start=True, stop=True)
            nc.vector.tensor_copy(bv_sb[:], pbv[:, 0:D])
            ppad = pso.tile([128, D + 2], F32, tag="po")
            nc.tensor.matmul(ppad[:, 0:128], ones_sb[:], sm_sb[:, D:D + 128],
                             start=True, stop=True)
            nc.vector.tensor_copy(vaug_sb[:, :, D:D + 2], ppad[:, 0:64])

            def emit_scores(qb, kt2, ets_list):
                """One [128,1024] scores tile (key tiles 2*kt2, 2*kt2+1) of
                query block qb, exp'd on ACT into a bf16 et tile."""
                ps = pss.tile([128, 1024], F32, tag="pss", name=f"ps_{qb}_{kt2}")
                for h in range(2):
                    kt = 2 * kt2 + h
                    for ec in range(2):
                        nc.tensor.matmul(
                            ps[:, 512 * h:512 * (h + 1)],
                            kt_sb[:, ec, 128 * kt:128 * (kt + 1)],
                            qt_sb[:, ec, 512 * qb:512 * (qb + 1)],
                            start=(ec == 0), stop=(ec == 1))
                et2 = ets.tile([128, 1024], ET_DT, tag="et",
                               name=f"et_{qb}_{kt2}")
                nc.scalar.activation(et2[:], ps[:], EXP)
                ets_list.append(et2)

            # ---- projections, with the scores of query-block 0 interleaved
            # (scores for key tiles of blk b-1 are emitted during blk b).
            # Elementwise split: DVE = K bias + V copies, ACT = Q bias + exps.
            ets0 = []
            for blk in range(8):
                # V[k,e] = sum_d xT[d,k] * wvT[d,e]  (no bias dep: first)
                # two k-chunks share one PSUM tile so the DVE copy is 512 wide
                for kc2 in range(2 * blk, 2 * blk + 2):
                    pv = pp.tile([128, 512], F32, tag="pp")
                    for h in range(2):
                        kc = 2 * kc2 + h
                        for dc in range(2):
                            nc.tensor.matmul(
                                pv[:, 256 * h:256 * (h + 1)],
                                xt_sb[:, dc, 128 * kc:128 * (kc + 1)],
                                w_sb[:, dc, 0:D],
                                start=(dc == 0), stop=(dc == 1))
                    nc.vector.tensor_copy(
                        vaug_sb[:, 2 * kc2:2 * kc2 + 2, 0:D],
                        pv[:].rearrange("p (a b) -> p a b", a=2))
                # QT[e,q]: queries are xT columns 0:SQ (blocks 0..3)
                if blk < 4:
                    for ec in range(2):
                        pq = pp.tile([128, 512], F32, tag="pp")
                        for dc in range(2):
                            nc.tensor.matmul(
                                pq[:],
                                w_sb[:, dc, D + 128 * ec:D + 128 * (ec + 1)],
                                xt_sb[:, dc, 512 * blk:512 * (blk + 1)],
                                start=(dc == 0), stop=(dc == 1))
                        nc.scalar.activation(
                            qt_sb[:, ec, 512 * blk:512 * (blk + 1)], pq[:],
                            mybir.ActivationFunctionType.Identity,
                            bias=bq_sb[:, ec:ec + 1])
                # KT[e,k] = sum_d wkT[d,e] * xT[d,k]  (+bk)
                for ec in range(2):
                    pk = pp.tile([128, 512], F32, tag="pp")
                    for dc in range(2):
                        nc.tensor.matmul(
                            pk[:],
                            w_sb[:, dc, 2 * D + 128 * ec:2 * D + 128 * (ec + 1)],
                            xt_sb[:, dc, 512 * blk:512 * (blk + 1)],
                            start=(dc == 0), stop=(dc == 1))
                    nc.vector.tensor_scalar_add(
                        kt_sb[:, ec, 512 * blk:512 * (blk + 1)], pk[:],
                        bk_sb[:, ec:ec + 1])
                # scores of query-block 0 over the key tiles of blk-1
                if blk >= 1:
                    for kt2 in range(2 * (blk - 1), 2 * blk):
                        emit_scores(0, kt2, ets0)

            # ---- attention: scores(qb) interleaved with AV(qb-1) ----
            def emit_av(ets_prev, qb_prev, qs, kc_lo, kc_hi, po):
                """AV matmuls for query-sub-block qs of qb_prev over key
                chunks [kc_lo, kc_hi); accumulate chain into po."""
                for kc in range(kc_lo, kc_hi):
                    et2 = ets_prev[kc // 2]
                    nc.tensor.matmul(
                        po[:],
                        et2[:, 512 * (kc % 2) + 128 * qs:
                            512 * (kc % 2) + 128 * (qs + 1)],
                        vaug_sb[:, kc, :],
                        start=(kc == 0), stop=(kc == N_KT - 1))

            def drain_av(qb_prev, qs, po):
                rec = small.tile([128, 1], F32, tag="rec")
                nc.vector.reciprocal(rec[:], po[:, D:D + 1])
                ot = outs.tile([128, D], F32, tag="ot")
                nc.vector.scalar_tensor_tensor(
                    ot[:], po[:, 0:D], rec[:], bv_sb[:],
                    mybir.AluOpType.mult, mybir.AluOpType.add)
                q0 = (qb_prev * 4 + qs) * 128
                nc.sync.dma_start(out[q0:q0 + 128, :], ot[:])

            prev_ets = ets0
            for qb in range(N_QB):
                cur_ets = ets0 if qb == 0 else []
                po = None
                for kt2 in range(N_KT // 2):
                    # qb 0's kt2 0..13 were already emitted inside the
                    # projection loop (blk 1..7)
                    if qb > 0 or kt2 >= 14:
                        emit_scores(qb, kt2, cur_ets)
                    if qb >= 1:
                        qs = kt2 // 4
                        if kt2 % 4 == 0:
                            po = pso.tile([128, D + 2], F32, tag="po")
                        emit_av(prev_ets, qb - 1, qs,
                                8 * (kt2 % 4), 8 * (kt2 % 4) + 8, po)
                        if kt2 % 4 == 3:
                            drain_av(qb - 1, qs, po)
                prev_ets = cur_ets

            # tail: AV for the last query block (PE-dense, nothing to overlap)
            for qs in range(4):
                po = pso.tile([128, D + 2], F32, tag="po")
                emit_av(prev_ets, N_QB - 1, qs, 0, N_KT, po)
                drain_av(N_QB - 1, qs, po)


_NC = None


def _get_nc():
    global _NC
    if _NC is None:
        _NC = _build()
    return _NC


def round_f32r(x: np.ndarray) -> np.ndarray:
    """Round fp32 to fp32r (12-bit mantissa) to match on-chip rounding."""
    u = x.astype(np.float32).view(np.uint32)
    u = ((u.astype(np.uint64) + 0x400) & 0xFFFFF800).astype(np.uint32)
    return u.view(np.float32)


def make_in_maps(x, Wq, bq, Wk, bk, Wv, bv):
    x = np.asarray(x, dtype=np.float32)
    Wq = np.asarray(Wq, dtype=np.float32)
    bq = np.asarray(bq, dtype=np.float32)
    Wk = np.asarray(Wk, dtype=np.float32)
    bk = np.asarray(bk, dtype=np.float32)
    Wv = np.asarray(Wv, dtype=np.float32)
    bv = np.asarray(bv, dtype=np.float32)

    scale = 1.0 / np.sqrt(np.float32(D))
    wall = round_f32r(np.concatenate(
        [Wv.T, Wq.T * scale, Wk.T], axis=1))  # [d_in, 3*e_out]
    bq2 = np.ascontiguousarray((bq * scale).reshape(2, 128).T)
    bk2 = np.ascontiguousarray(bk.reshape(2, 128).T)
    vpad_row = np.zeros(64, dtype=np.float32)
    vpad_row[0::2] = 1.0
    smalls = np.concatenate([bv, vpad_row, np.zeros(64, np.float32)]).reshape(1, D + 128)
    smalls = np.ascontiguousarray(smalls.astype(np.float32))

    in_maps = []
    for c in range(N_CORES):
        b, h = divmod(c, 2)
        xt = x[b].T  # [D, S]
        # rotate this core's query half to columns 0:SQ (k-perm is invariant)
        xt = np.concatenate([xt[:, h * SQ:(h + 1) * SQ],
                             xt[:, (1 - h) * SQ:(2 - h) * SQ]], axis=1)
        in_maps.append({
            "xT": round_f32r(np.ascontiguousarray(xt)),
            "wall": wall,
            "bq2": bq2, "bk2": bk2, "smalls": smalls,
        })
    return in_maps


def kernel(x, Wq, bq, Wk, bk, Wv, bv):
    in_maps = make_in_maps(x, Wq, bq, Wk, bk, Wv, bv)
    nc = _get_nc()
    res = run_bass_kernel_spmd(nc, in_maps, core_ids=list(range(N_CORES)))

    full = np.empty((B, S, D), dtype=np.float32)
    for c in range(N_CORES):
        b, h = divmod(c, 2)
        full[b, h * SQ:(h + 1) * SQ, :] = res.results[c]["out"]
    return full


# revision 9
# speedup vs baseline: 8.6412x; 4.5037x over previous
"""Trainium2 Bass kernel for fused single-head attention.

Problem: x:(4,4096,256) f32; q/k/v = x@W*.T + b*; out = softmax(q@k.T/16) @ v.

Sharding: 8 cores = 4 batches x 2 query-halves. Each core holds full K/V for
its batch (recomputed per core-pair; proj is cheap) and half the queries.

Algebra (v3): fold Wq into the key side on the host. With s = 1/sqrt(D):
    s*q_i.k_j = s*x_j (Wk^T Wq) x_i^T  +  g_j  +  h_i
where g_j = s*x_j.(Wk^T bq) is per-KEY and h_i is per-QUERY. h_i is constant
along the softmax axis, so it cancels; g_j is folded in as a per-key
multiplicative factor e^{g_j} applied to the augmented V rows (including the
denominator ones-column), which the softmax division renormalizes exactly.
This removes the whole Q projection. e^g is a tiny host-precomputed input
(like the packed biases).

Per-core algorithm (matmuls in float32r: full PE rate at N>=256):
  xT[d,s]   : x[b].T with this core's query half rotated to columns 0:2048
              (a key-permutation is softmax-invariant, so one SPMD program
              serves both halves).
  A1[e,k]   = (s*M)^T @ xT           (M = Wk^T Wq, host-packed into wall)
  V[k,e]    = xT.T @ wvT             (bv folded in after softmax)
  vaug[k,:] = [V[k,:] | e^{g_k} | 0] (k-chunked, bf16)
  ST[k,q]   = A1.T[:,k] @ xT[:,q]    (scores', k on partitions)
  ET[k,q]   = exp(ST)
  acc[q,e+1]= ET.T @ vaug            (col D -> softmax denominators)
  out[q,e]  = acc[:, :256] / acc[:, 256] + bv

Schedule: PE is the bottleneck engine, so the emission order keeps it dense:
  - AV (output) matmuls of query-block qb-1 interleave between the scores
    matmuls of qb, so the PE never stalls on the ACT exp drain of the
    scores PSUM tiles;
  - exp runs on 1024-wide PSUM tiles (2 key-tiles per instruction);
  - the first 7 scores tiles of query-block 0 are emitted inside the
    projection loop;
  - elementwise work is split ACT/DVE to keep both under the PE pace;
  - input DMA streams on the sync + gpsimd rings (both otherwise idle).
"""

import numpy as np

import concourse.bass as bass
import concourse.mybir as mybir
import concourse.tile as tile
from concourse import bacc
from concourse.bass_utils import run_bass_kernel_spmd

B, S, D = 4, 4096, 256
N_CORES = 8
SQ = S // 2          # queries per core
F32 = mybir.dt.float32
F32R = mybir.dt.float32r
EXP = mybir.ActivationFunctionType.Exp
COPY = mybir.ActivationFunctionType.Copy
BF16 = mybir.dt.bfloat16
ET_DT = BF16

N_QB = SQ // 512     # 4 query blocks of 512
N_KT = S // 128      # 32 key tiles of 128
N_PROJ_SC = 7        # qb0 scores pairs emitted inside the projection loop


def _build(repeats=1):
    nc = bacc.Bacc("TRN2", target_bir_lowering=False, debug=False,
                   enable_asserts=False, num_devices=N_CORES)

    xT = nc.dram_tensor("xT", [D, S], F32R, kind="ExternalInput").ap()
    # packed [WvT | (s*M)] along columns: [D, 2*D]
    wall = nc.dram_tensor("wall", [D, 2 * D], F32R, kind="ExternalInput").ap()
    # e^{g_k}, k-chunked: [128, N_KT]
    eg = nc.dram_tensor("eg", [128, N_KT], F32, kind="ExternalInput").ap()
    smalls = nc.dram_tensor("smalls", [1, D], F32, kind="ExternalInput").ap()
    out = nc.dram_tensor("out", [SQ, D], F32, kind="ExternalOutput").ap()

    with tile.TileContext(nc) as tc:
        for _rep in range(repeats):
            _emit_body(nc, tc, _rep, xT, wall, eg, smalls, out)

    nc.compile()
    return nc


def _emit_body(nc, tc, rep, xT, wall, eg, smalls, out):
        with (
            tc.tile_pool(name=f"persist{rep}", bufs=1) as persist,
            tc.tile_pool(name=f"ets{rep}", bufs=34) as ets,
            tc.tile_pool(name=f"outs{rep}", bufs=3) as outs,
            tc.tile_pool(name=f"small{rep}", bufs=4) as small,
            tc.tile_pool(name=f"pp{rep}", bufs=2, space="PSUM") as pp,
            tc.tile_pool(name=f"pss{rep}", bufs=2, space="PSUM") as pss,
            tc.tile_pool(name=f"pso{rep}", bufs=2, space="PSUM") as pso,
        ):
            # ---- persistent SBUF ----
            xt_sb = persist.tile([128, 2, S], F32R)       # x[b].T, d-chunked
            w_sb = persist.tile([128, 2, 2 * D], F32R)    # [wv|sM], d-chunked
            a1_sb = persist.tile([128, 2, S], F32R)       # A1^T, e-chunked
            # [V | e^g | 0] k-chunked
            vaug_sb = persist.tile([128, N_KT, D + 2], ET_DT)
            eg_sb = persist.tile([128, N_KT], F32)
            bv_sb = persist.tile([128, D], F32)
            sm_sb = persist.tile([1, D], F32)

            # ---- input DMA: sync ring = wv + even x blocks; gpsimd ring =
            # small tensors, sM, odd x blocks. Both engines are otherwise
            # idle. The first x block streams in 128-col pieces so the first
            # V matmul can start early.
            rs, rg = nc.sync, nc.gpsimd
            rg.dma_start(eg_sb[:], eg[:])
            rg.dma_start(sm_sb[:], smalls[:])
            for c in range(2):
                rs.dma_start(w_sb[:, c, 0:D], wall[128 * c:128 * (c + 1), 0:D])
            for kc in range(4):
                for c in range(2):
                    rs.dma_start(
                        xt_sb[:, c, 128 * kc:128 * (kc + 1)],
                        xT[128 * c:128 * (c + 1), 128 * kc:128 * (kc + 1)])
            for c in range(2):
                rg.dma_start(w_sb[:, c, D:2 * D],
                             wall[128 * c:128 * (c + 1), D:2 * D])
            for blk in range(2, 8, 2):
                for c in range(2):
                    rs.dma_start(
                        xt_sb[:, c, 512 * blk:512 * (blk + 1)],
                        xT[128 * c:128 * (c + 1), 512 * blk:512 * (blk + 1)])
            for blk in range(1, 8, 2):
                for c in range(2):
                    rg.dma_start(
                        xt_sb[:, c, 512 * blk:512 * (blk + 1)],
                        xT[128 * c:128 * (c + 1), 512 * blk:512 * (blk + 1)])

            # vaug pad/denominator columns: e^g then a zero column
            nc.vector.tensor_copy(vaug_sb[:, :, D:D + 1],
                                  eg_sb[:].rearrange("p (a b) -> p a b", b=1))
            nc.vector.memset(vaug_sb[:, :, D + 1:D + 2], 0.0)

            # PE warm-up: dummy matmuls on a memset tile (no DMA dep) while
            # the input stream lands -- flips the HAM clock-gate to 2.4 GHz
            # and keeps the PE busy through the initial load.
            ones_sb = persist.tile([1, 128], F32)
            nc.vector.memset(ones_sb[:], 1.0)
            warm = pp.tile([128, 512], F32, tag="pp")
            for _w in range(6):
                nc.tensor.matmul(warm[:, 0:128], ones_sb[:],
                                 ones_sb[:], start=True, stop=True)

            # broadcast bv across partitions via a K=1 matmul against a ones
            # row (smalls is at the head of the gpsimd ring)
            pbv = pso.tile([128, D + 2], F32, tag="po")
            nc.tensor.matmul(pbv[:, 0:D], ones_sb[:], sm_sb[:],
                             start=True, stop=True)
            nc.vector.tensor_copy(bv_sb[:], pbv[:, 0:D])

            def emit_scores(qb, kt2, ets_list):
                """One [128,1024] scores tile (key tiles 2*kt2, 2*kt2+1) of
                query block qb, exp'd on ACT into a bf16 et tile."""
                ps = pss.tile([128, 1024], F32, tag="pss", name=f"ps_{qb}_{kt2}")
                for h in range(2):
                    kt = 2 * kt2 + h
                    for ec in range(2):
                        nc.tensor.matmul(
                            ps[:, 512 * h:512 * (h + 1)],
                            a1_sb[:, ec, 128 * kt:128 * (kt + 1)],
                            xt_sb[:, ec, 512 * qb:512 * (qb + 1)],
                            start=(ec == 0), stop=(ec == 1))
                et2 = ets.tile([128, 1024], ET_DT, tag="et",
                               name=f"et_{qb}_{kt2}")
                nc.scalar.activation(et2[:], ps[:], EXP)
                ets_list.append(et2)

            # ---- projections, with the first scores of query-block 0
            # interleaved (one pair per blk from blk 1 on).
            # Elementwise split: DVE = 1 V copy + A1 copies, ACT = 1 V copy.
            ets0 = []
            for blk in range(8):
                # V[k,e] = sum_d xT[d,k] * wvT[d,e], then scaled by e^{g_k}
                # per key row during the PSUM->SBUF copy (per-partition scale).
                # Two k-chunks share one PSUM tile; copies alternate DVE/ACT.
                for i, kc2 in enumerate(range(2 * blk, 2 * blk + 2)):
                    pv = pp.tile([128, 512], F32, tag="pp")
                    for h in range(2):
                        kc = 2 * kc2 + h
                        for dc in range(2):
                            nc.tensor.matmul(
                                pv[:, 256 * h:256 * (h + 1)],
                                xt_sb[:, dc, 128 * kc:128 * (kc + 1)],
                                w_sb[:, dc, 0:D],
                                start=(dc == 0), stop=(dc == 1))
                    for h in range(2):
                        kc = 2 * kc2 + h
                        dst = vaug_sb[:, kc, 0:D]
                        src = pv[:, 256 * h:256 * (h + 1)]
                        egc = eg_sb[:, kc:kc + 1]
                        if i == 0:
                            nc.vector.tensor_scalar_mul(dst, src, egc)
                        else:
                            nc.scalar.activation(dst, src, COPY, scale=egc)
                # A1[e,k] = sum_d (s*M)[d,e] * xT[d,k]
                for ec in range(2):
                    pk = pp.tile([128, 512], F32, tag="pp")
                    for dc in range(2):
                        nc.tensor.matmul(
                            pk[:],
                            w_sb[:, dc, D + 128 * ec:D + 128 * (ec + 1)],
                            xt_sb[:, dc, 512 * blk:512 * (blk + 1)],
                            start=(dc == 0), stop=(dc == 1))
                    nc.vector.tensor_copy(
                        a1_sb[:, ec, 512 * blk:512 * (blk + 1)], pk[:])
                # one scores pair of query-block 0 per blk (keys of blk-1)
                if 1 <= blk <= N_PROJ_SC:
                    emit_scores(0, blk - 1, ets0)

            # ---- attention: scores(qb) interleaved with AV(qb-1) ----
            def emit_av(ets_prev, qb_prev, qs, kc_lo, kc_hi, po):
                """AV matmuls for query-sub-block qs of qb_prev over key
                chunks [kc_lo, kc_hi); accumulate chain into po."""
                for kc in range(kc_lo, kc_hi):
                    et2 = ets_prev[kc // 2]
                    nc.tensor.matmul(
                        po[:],
                        et2[:, 512 * (kc % 2) + 128 * qs:
                            512 * (kc % 2) + 128 * (qs + 1)],
                        vaug_sb[:, kc, :],
                        start=(kc == 0), stop=(kc == N_KT - 1))

            def drain_av(qb_prev, qs, po):
                rec = small.tile([128, 1], F32, tag="rec")
                nc.vector.reciprocal(rec[:], po[:, D:D + 1])
                ot = outs.tile([128, D], F32, tag="ot")
                nc.vector.scalar_tensor_tensor(
                    ot[:], po[:, 0:D], rec[:], bv_sb[:],
                    mybir.AluOpType.mult, mybir.AluOpType.add)
                q0 = (qb_prev * 4 + qs) * 128
                nc.sync.dma_start(out[q0:q0 + 128, :], ot[:])

            prev_ets = ets0
            for qb in range(N_QB):
                cur_ets = ets0 if qb == 0 else []
                po = None
                for kt2 in range(N_KT // 2):
                    # qb 0's first pairs were emitted inside the proj loop
                    if qb > 0 or kt2 >= N_PROJ_SC:
                        emit_scores(qb, kt2, cur_ets)
                    if qb >= 1:
                        qs = kt2 // 4
                        if kt2 % 4 == 0:
                            po = pso.tile([128, D + 2], F32, tag="po")
                        emit_av(prev_ets, qb - 1, qs,
                                8 * (kt2 % 4), 8 * (kt2 % 4) + 8, po)
                        if kt2 % 4 == 3:
                            drain_av(qb - 1, qs, po)
                prev_ets = cur_ets

            # tail: AV for the last query block (PE-dense, nothing to overlap)
            for qs in range(4):
                po = pso.tile([128, D + 2], F32, tag="po")
                emit_av(prev_ets, N_QB - 1, qs, 0, N_KT, po)
                drain_av(N_QB - 1, qs, po)


_NC = None


def _get_nc():
    global _NC
    if _NC is None:
        _NC = _build()
    return _NC


def round_f32r(x: np.ndarray) -> np.ndarray:
    """Round fp32 to fp32r (12-bit mantissa) to match on-chip rounding."""
    u = x.astype(np.float32).view(np.uint32)
    u = ((u.astype(np.uint64) + 0x400) & 0xFFFFF800).astype(np.uint32)
    return u.view(np.float32)


def make_in_maps(x, Wq, bq, Wk, bk, Wv, bv):
    x = np.asarray(x, dtype=np.float32)
    Wq = np.asarray(Wq, dtype=np.float32)
    bq = np.asarray(bq, dtype=np.float32)
    Wk = np.asarray(Wk, dtype=np.float32)
    bk = np.asarray(bk, dtype=np.float32)
    Wv = np.asarray(Wv, dtype=np.float32)
    bv = np.asarray(bv, dtype=np.float32)

    s = np.float32(1.0 / np.sqrt(np.float32(D)))
    # M-trick: scores' = x (s Wk^T Wq) x^T + g, g = x.(s Wk^T bq); the
    # per-query term x.(s Wq^T bk) + s bq.bk is softmax-invariant and dropped.
    M = (Wk.T @ Wq) * s                 # [d_in, e']
    gw = (Wk.T @ bq) * s                # [d_in]
    wall = round_f32r(np.concatenate([Wv.T, M], axis=1))  # [d_in, 2*D]
    smalls = np.ascontiguousarray(bv.reshape(1, D))

    in_maps = []
    for c in range(N_CORES):
        b, h = divmod(c, 2)
        xt = x[b].T  # [D, S]
        # rotate this core's query half to columns 0:SQ (k-perm is invariant)
        xt = np.concatenate([xt[:, h * SQ:(h + 1) * SQ],
                             xt[:, (1 - h) * SQ:(2 - h) * SQ]], axis=1)
        xt = round_f32r(np.ascontiguousarray(xt))
        g = xt.T @ gw                    # per-key bias, local key order
        eg = np.ascontiguousarray(
            np.exp(g).reshape(N_KT, 128).T.astype(np.float32))
        in_maps.append({
            "xT": xt,
            "wall": wall,
            "eg": eg, "smalls": smalls,
        })
    return in_maps


def kernel(x, Wq, bq, Wk, bk, Wv, bv):
    in_maps = make_in_maps(x, Wq, bq, Wk, bk, Wv, bv)
    nc = _get_nc()
    res = run_bass_kernel_spmd(nc, in_maps, core_ids=list(range(N_CORES)))

    full = np.empty((B, S, D), dtype=np.float32)
    for c in range(N_CORES):
        b, h = divmod(c, 2)
        full[b, h * SQ:(h + 1) * SQ, :] = res.results[c]["out"]
    return full
